# revision 1
# baseline (speedup 1.0000x reference)
"""Trainium2 Bass kernel for nn_Attention_5669356831317.

Dense causal multi-head attention with rotary embeddings on q/k/v:
    qkv = x @ W_qkv ; rotary(q,k,v) ; softmax(causal(q k^T / sqrt(dh))) v ; out @ W_out + b_out

Sharding over 8 NeuronCores:
  - Heads are tensor-parallel: 16 heads / 8 cores = 2 heads per core.
    Each core computes qkv^T for its 2 heads (K=1024 matmul against x^T),
    applies rotary (rotate-half folded into a PE matmul with a signed
    permutation matrix), and runs causal attention for its 8 (batch, head)
    units in a transposed-scores layout: S^T[key, query] so the exp output is
    directly the lhsT-ready P^T, and the softmax denominator comes for free
    from a ones-column appended to V in the P^T @ V matmul.
  - A per-batch AllToAll reshards from head-parallel to row-parallel: each
    core ends with all 1024 inner dims for its 256 rows of each batch, then
    computes its row slice of the output projection (full W_out) + bias.
  - Work is software-pipelined across batches (qkv(b+1) overlaps attention(b)
    overlaps collective(b-1) overlaps projection(b-1)).
  - Host reassembles the row slices.

All matmuls run in float32r (full-rate fp32 PE mode, ~1e-4 relative error).
"""

import numpy as np

import concourse.bass as bass
import concourse.bacc as bacc
import concourse.tile as tile
import concourse.mybir as mybir
from concourse.bass_utils import run_bass_kernel_spmd

B, N, D = 4, 2048, 1024
H, DH = 16, 64
NCORES = 8
ROWS = B * N  # 8192
RPB = N // NCORES  # 256 output rows per (core, batch)
SCALE = DH**-0.5

f32 = mybir.dt.float32
f32r = mybir.dt.float32r
AF = mybir.ActivationFunctionType

_CACHE = {}


def _build_nc(single=False):
    nc = bacc.Bacc(
        "TRN2",
        target_bir_lowering=False,
        debug=False,
        num_devices=1 if single else NCORES,
    )

    xT_d = nc.dram_tensor("xT", [16, 128, 8, 512], f32r, kind="ExternalInput")
    wqkv_d = nc.dram_tensor("wqkv", [128, 8, 3, 128], f32r, kind="ExternalInput")
    cosT_d = nc.dram_tensor("cosT", [128, N], f32, kind="ExternalInput")
    sinT_d = nc.dram_tensor("sinT", [128, N], f32, kind="ExternalInput")
    rblk_d = nc.dram_tensor("rblk", [128, 128], f32r, kind="ExternalInput")
    wout_d = nc.dram_tensor("wout", [128, 8, D], f32r, kind="ExternalInput")
    bias_d = nc.dram_tensor("bias", [1, D], f32, kind="ExternalInput")
    cmask_d = nc.dram_tensor("cmask", [128, 128], f32, kind="ExternalInput")
    cmask256_d = nc.dram_tensor("cmask256", [128, 256], f32, kind="ExternalInput")
    ident128_d = nc.dram_tensor("ident128", [128, 128], f32, kind="ExternalInput")

    out_d = nc.dram_tensor("out_rows", [B, RPB, D], f32, kind="ExternalOutput")

    with tile.TileContext(nc) as tc:
        with (
            tc.tile_pool(name="const", bufs=1) as const_pool,
            tc.tile_pool(name="big", bufs=1) as big_pool,
            tc.tile_pool(name="xp", bufs=2) as x_pool,
            tc.tile_pool(name="work", bufs=2) as work_pool,
            tc.tile_pool(name="ptp", bufs=3) as pt_pool,
            tc.tile_pool(name="otfp", bufs=1) as otf_pool,
            tc.tile_pool(name="tinyp", bufs=1) as tiny_pool,
            tc.tile_pool(name="ps", bufs=2, space="PSUM") as ps_pool,
            tc.tile_pool(name="psot", bufs=2, space="PSUM") as psot_pool,
            tc.tile_pool(name="dram", bufs=1, space="DRAM") as dram_pool,
        ):
            # ---- constants (scalar=ACT HWDGE ring; sync=SP ring) ----
            # wqkv first: phase1's first matmuls gate on it
            wqkv_sb = const_pool.tile([128, 8, 3, 128], f32r)
            nc.scalar.dma_start(wqkv_sb[:], wqkv_d[:])
            rblk_sb = const_pool.tile([128, 128], f32r)
            nc.scalar.dma_start(rblk_sb[:], rblk_d[:])
            cosT_sb = const_pool.tile([128, N], f32)
            nc.scalar.dma_start(cosT_sb[:], cosT_d[:])
            sinT_sb = const_pool.tile([128, N], f32)
            nc.scalar.dma_start(sinT_sb[:], sinT_d[:])
            ident128_f = const_pool.tile([128, 128], f32)
            nc.scalar.dma_start(ident128_f[:], ident128_d[:])
            ident128_r = const_pool.tile([128, 128], f32r)
            nc.vector.tensor_copy(ident128_r[:], ident128_f[:])
            cmask_sb = const_pool.tile([128, 128], f32)
            nc.scalar.dma_start(cmask_sb[:], cmask_d[:])
            cmask256_sb = const_pool.tile([128, 256], f32)
            nc.scalar.dma_start(cmask256_sb[:], cmask256_d[:])
            ones_f = const_pool.tile([128, 1], f32)
            nc.vector.memset(ones_f[:], 1.0)
            # deferred: wout/bias DMAs are emitted after phase1(1) (see below)
            wout_sb = const_pool.tile([128, 8, D], f32r)
            bias_rep = const_pool.tile([128, D], f32)

            # ---- per-batch activations, rotated through 3 slots each ----
            qT_b, kT_b, vne_b = [], [], []
            for b in range(B):
                qT = big_pool.tile([128, N], f32r, name=f"qT_{b}", tag="qT", bufs=3)
                kT = big_pool.tile([128, N], f32r, name=f"kT_{b}", tag="kT", bufs=3)
                vne = big_pool.tile(
                    [128, 2, 16, 65], f32r, name=f"vne_{b}", tag="vne", bufs=3
                )
                nc.vector.tensor_copy(
                    vne[:, :, :, 64:65], ones_f[:].to_broadcast((128, 2, 16, 1))
                )
                qT_b.append(qT)
                kT_b.append(kT)
                vne_b.append(vne)

            a2a_in_b = [
                dram_pool.tile([8, 128, RPB], f32r, name=f"a2a_in_{b}")
                for b in range(B)
            ]
            a2a_out_b = [
                dram_pool.tile([8, 128, RPB], f32r, name=f"a2a_out_{b}")
                for b in range(B)
            ]
            # last batch exchanges per q-half so the first half's collective
            # fires while the second half's attention still runs
            a2a_in3 = [
                dram_pool.tile([8, 128, 128], f32r, name=f"a2a_in3_{qh}")
                for qh in range(2)
            ]
            a2a_out3 = [
                dram_pool.tile([8, 128, 128], f32r, name=f"a2a_out3_{qh}")
                for qh in range(2)
            ]

            def phase1_gen(b):
                """qkv^T + rotary for batch b; yields after each 512-chunk."""
                for jj in range(4):  # 512-wide chunks within the batch
                    j = b * 4 + jj
                    cosc = cosT_sb[:, jj * 512 : (jj + 1) * 512]
                    sinc = sinT_sb[:, jj * 512 : (jj + 1) * 512]
                    acA = ps_pool.tile([128, 1024], f32, tag="ps", name="acA")
                    acB = ps_pool.tile([128, 1024], f32, tag="ps", name="acB")
                    # accumulation regions: q=acA[0:512], k=acA[512:1024], v=acB[0:512]
                    regions = [acA[:, 0:512], acA[:, 512:1024], acB[:, 0:512]]
                    x8 = x_pool.tile([128, 8, 512], f32r, tag="x8")
                    if j == 0:
                        # split the very first chunk across both rings so the
                        # first matmuls start as early as possible
                        nc.sync.dma_start(x8[:, 0:4, :], xT_d[0, :, 0:4, :])
                        nc.scalar.dma_start(x8[:, 4:8, :], xT_d[0, :, 4:8, :])
                    else:
                        eng = nc.sync if j % 2 == 0 else nc.scalar
                        eng.dma_start(x8[:], xT_d[j])
                    for k in range(8):
                        for m in range(3):
                            nc.tensor.matmul(
                                regions[m],
                                wqkv_sb[:, k, m, :],
                                x8[:, k, :],
                                start=(k == 0),
                                stop=(k == 7),
                            )
                    vrot = None
                    for m in range(3):  # q, k, v
                        raw = work_pool.tile([128, 512], f32r, tag="raw")
                        nc.scalar.copy(raw[:], regions[m])  # evacuate+round (ACT)
                        rot = acB[:, 512:1024]  # rotate-half scratch bank
                        nc.tensor.matmul(rot, rblk_sb[:], raw[:], start=True, stop=True)
                        tmp = work_pool.tile([128, 512], f32, tag="tmp")
                        nc.vector.tensor_mul(tmp[:], rot, sinc)
                        if m < 2:
                            dest = (qT_b[b] if m == 0 else kT_b[b])[
                                :, jj * 512 : (jj + 1) * 512
                            ]
                            nc.gpsimd.tensor_mul(dest, raw[:], cosc)
                            nc.vector.tensor_add(dest, dest, tmp[:])
                        else:
                            vrot = work_pool.tile([128, 512], f32r, tag="vrot")
                            nc.gpsimd.tensor_mul(vrot[:], raw[:], cosc)
                            nc.vector.tensor_add(vrot[:], vrot[:], tmp[:])
                    # transpose v' into normal layout; each [128,128] transpose
                    # yields both heads' [n, dh] blocks side by side
                    vt_ps = ps_pool.tile([128, 1024], f32r, tag="ps", name="vt_ps")
                    for t in range(4):
                        nc.tensor.transpose(
                            vt_ps[:, t * 256 : t * 256 + 128],
                            vrot[:, t * 128 : (t + 1) * 128],
                            ident128_r[:],
                        )
                    for t in range(4):
                        jb = jj * 4 + t
                        nc.vector.tensor_copy(
                            vne_b[b][:, :, jb, 0:64],
                            vt_ps[:, t * 256 : t * 256 + 128].rearrange(
                                "p (h d) -> p h d", h=2
                            ),
                        )
                    yield

            def attn_gen(b, qh_hook=None):
                """Causal attention for batch b; both head-halves advance
                together so their K=64 scores matmuls occupy disjoint PE
                row-groups concurrently. Yields after each jb step."""
                for qh in range(2):
                    qbase = qh * 1024
                    OTs = [
                        psot_pool.tile([65, 1024], f32, tag="ot", name=f"OT_{hh}")
                        for hh in range(2)
                    ]
                    jb_max = 8 * qh + 7
                    for jb in range(jb_max + 1):
                        w0 = max(0, jb * 128 - qbase)
                        # fp32r matmuls run 4x slower below 256 columns: widen
                        # a 128-wide diagonal partial to 256 and zero the extra
                        # 128 invalid columns with the extended causal mask
                        widen = jb * 128 > qbase and (jb * 128 - qbase) % 512 == 384
                        w0e = w0 - 128 if widen else w0

                        def _ranges():
                            for sc in range(2):
                                clo = qbase + sc * 512
                                chi = clo + 512
                                lo = max(clo, jb * 128)
                                if lo >= chi:
                                    continue
                                if chi - lo == 128:
                                    lo -= 128
                                yield sc, lo, chi

                        sts = [
                            ps_pool.tile([128, 1024], f32, tag="ps", name=f"st_{hh}")
                            for hh in range(2)
                        ]
                        # alternate head-halves so consecutive matmuls land on
                        # different PE row-groups (base partitions 0 / 64)
                        for sc, lo, chi in _ranges():
                            for hh in range(2):
                                hsl = slice(hh * 64, (hh + 1) * 64)
                                nc.tensor.matmul(
                                    sts[hh][:, lo - qbase : chi - qbase],
                                    kT_b[b][hsl, jb * 128 : (jb + 1) * 128],
                                    qT_b[b][hsl, lo:chi],
                                    start=True,
                                    stop=True,
                                )
                        for hh in range(2):
                            pt = pt_pool.tile([128, 1024], f32r, tag="pt")
                            nc.scalar.activation(
                                pt[:, w0e:1024], sts[hh][:, w0e:1024], AF.Exp, scale=SCALE
                            )
                            if jb * 128 >= qbase:
                                # zero below-diagonal keys (and the widened
                                # invalid columns, if any)
                                if widen:
                                    nc.vector.tensor_mul(
                                        pt[:, w0e : w0e + 256],
                                        pt[:, w0e : w0e + 256],
                                        cmask256_sb[:],
                                    )
                                else:
                                    nc.vector.tensor_mul(
                                        pt[:, w0 : w0 + 128],
                                        pt[:, w0 : w0 + 128],
                                        cmask_sb[:],
                                    )
                            vw = vne_b[b][:, hh, jb, :]
                            for sc, lo, chi in _ranges():
                                nc.tensor.matmul(
                                    OTs[hh][:, lo - qbase : chi - qbase],
                                    vw,
                                    pt[:, lo - qbase : chi - qbase],
                                    start=(jb == 0),
                                    stop=(jb == 8 * qh + 4 * sc + 3),
                                )
                        yield
                    # normalize by the ones-column sums, stage into qT_b[b]
                    for hh in range(2):
                        hsl = slice(hh * 64, (hh + 1) * 64)
                        gsl = slice(qbase, qbase + 1024)
                        rep = tiny_pool.tile([64, 1024], f32, tag="rep")
                        nc.vector.reciprocal(rep[0:1, :], OTs[hh][64:65, :])
                        nc.gpsimd.partition_broadcast(rep[:], rep[0:1, :], channels=64)
                        nc.vector.tensor_mul(
                            qT_b[b][hsl, gsl], OTs[hh][0:64, :], rep[:]
                        )
                    if qh_hook is not None:
                        qh_hook(qh)

            def stage(b):
                """Ship batch b's attention output through the AllToAll."""
                nc.sync.dma_start(
                    a2a_in_b[b][:].rearrange("t p r -> p t r"),
                    qT_b[b][:].rearrange("p (t r) -> p t r", t=8),
                )
                if single:
                    nc.sync.dma_start(a2a_out_b[b][:], a2a_in_b[b][:])
                else:
                    nc.gpsimd.collective_compute(
                        "AllToAll",
                        mybir.AluOpType.bypass,
                        replica_groups=[list(range(NCORES))],
                        ins=[a2a_in_b[b][:]],
                        outs=[a2a_out_b[b][:]],
                    )

            def stage3_half(qh):
                nc.sync.dma_start(
                    a2a_in3[qh][:].rearrange("t p r -> p t r"),
                    qT_b[3][:, qh * 1024 : (qh + 1) * 1024].rearrange(
                        "p (t r) -> p t r", t=8
                    ),
                )
                if single:
                    nc.sync.dma_start(a2a_out3[qh][:], a2a_in3[qh][:])
                else:
                    nc.gpsimd.collective_compute(
                        "AllToAll",
                        mybir.AluOpType.bypass,
                        replica_groups=[list(range(NCORES))],
                        ins=[a2a_in3[qh][:]],
                        outs=[a2a_out3[qh][:]],
                    )

            def proj_gen(b):
                """Output projection for this core's 256 rows of batch b, in
                self-contained per-row-chunk pieces so it can interleave into
                attention."""
                otf2 = otf_pool.tile([128, 8, RPB], f32r, tag="otf")
                if b == 3:
                    for qh in range(2):
                        nc.sync.dma_start(
                            otf2[:, :, qh * 128 : (qh + 1) * 128],
                            a2a_out3[qh][:].rearrange("i p r -> p i r"),
                        )
                else:
                    nc.sync.dma_start(
                        otf2[:], a2a_out_b[b][:].rearrange("i p r -> p i r")
                    )
                yield
                for rr in range(2):
                    ps = ps_pool.tile([128, 1024], f32, tag="ps", name=f"pp_{rr}")
                    for k in range(8):
                        for n_ in range(2):
                            nc.tensor.matmul(
                                ps[:, n_ * 512 : (n_ + 1) * 512],
                                otf2[:, k, rr * 128 : (rr + 1) * 128],
                                wout_sb[:, k, n_ * 512 : (n_ + 1) * 512],
                                start=(k == 0),
                                stop=(k == 7),
                            )
                    for n_ in range(2):
                        res = work_pool.tile([128, 512], f32, tag="tmp")
                        nc.vector.tensor_add(
                            res[:],
                            ps[:, n_ * 512 : (n_ + 1) * 512],
                            bias_rep[:, n_ * 512 : (n_ + 1) * 512],
                        )
                        nc.scalar.dma_start(
                            out_d[
                                b,
                                rr * 128 : (rr + 1) * 128,
                                n_ * 512 : (n_ + 1) * 512,
                            ],
                            res[:],
                        )
                    yield

            # software pipeline across batches: attention(b) is interleaved
            # with phase1(b+1) at (jb-step, chunk) granularity so the PE
            # absorbs the ACT exp-throughput deficit.
            def run_all(gen):
                for _ in gen:
                    pass

            def interleave(attn_g, p1_g, every=10):
                i = 0
                for _ in attn_g:
                    i += 1
                    if p1_g is not None and i % every == 0:
                        next(p1_g, None)
                if p1_g is not None:
                    run_all(p1_g)

            run_all(phase1_gen(0))
            run_all(phase1_gen(1))
            # projection weights arrive while attention runs
            nc.scalar.dma_start(wout_sb[:], wout_d[:])
            nc.scalar.dma_start(bias_rep[:], bias_d[:].to_broadcast((128, D)))
            interleave(attn_gen(0), phase1_gen(2))
            stage(0)
            interleave(attn_gen(1), phase1_gen(3))
            stage(1)
            run_all(proj_gen(0))
            interleave(attn_gen(2), proj_gen(1), every=8)
            stage(2)
            interleave(attn_gen(3, qh_hook=stage3_half), proj_gen(2), every=8)
            run_all(proj_gen(3))

    nc.compile()
    return nc


def _host_prep(x, rotary_pos_emb, W_qkv, W_out, b_out):
    x = np.asarray(x, dtype=np.float32)
    W_qkv = np.asarray(W_qkv, dtype=np.float32)
    W_out = np.asarray(W_out, dtype=np.float32)
    b_out = np.asarray(b_out, dtype=np.float32)
    rot = np.asarray(rotary_pos_emb, dtype=np.float32)

    xf = np.ascontiguousarray(x.reshape(ROWS, D))
    # [16, 128, 8, 512]: chunk j, partition p, k-chunk, col n -> xf[j*512+n, k*128+p]
    xT = np.ascontiguousarray(
        xf.reshape(16, 512, 8, 128).transpose(0, 3, 2, 1)
    )

    cos = np.cos(rot).T  # [64, 2048]
    sin = np.sin(rot).T
    cosT = np.ascontiguousarray(np.tile(cos, (2, 1)))
    sinT = np.ascontiguousarray(np.tile(sin, (2, 1)))

    # rotate_half as a matrix: (R t)[2i] = -t[2i+1], (R t)[2i+1] = t[2i]
    R64 = np.zeros((64, 64), np.float32)
    idx = np.arange(0, 64, 2)
    R64[idx, idx + 1] = -1.0
    R64[idx + 1, idx] = 1.0
    rblk = np.zeros((128, 128), np.float32)
    rblk[0:64, 0:64] = R64.T
    rblk[64:128, 64:128] = R64.T

    cmask = (np.arange(128)[:, None] <= np.arange(128)[None, :]).astype(np.float32)
    cmask256 = np.concatenate([np.zeros((128, 128), np.float32), cmask], axis=1)
    ident128 = np.eye(128, dtype=np.float32)

    # wout[p, k, o] = W_out[k*128+p, o]
    wout = np.ascontiguousarray(W_out.reshape(8, 128, D).transpose(1, 0, 2))
    bias = np.ascontiguousarray(b_out.reshape(1, D))

    in_maps = []
    for c in range(NCORES):
        hsl = slice(2 * c * 64, (2 * c + 2) * 64)
        Wq = W_qkv[:, 0 * D : 1 * D][:, hsl]
        Wk = W_qkv[:, 1 * D : 2 * D][:, hsl]
        Wv = W_qkv[:, 2 * D : 3 * D][:, hsl]
        arr = np.stack([Wq, Wk, Wv], axis=1)  # [1024, 3, 128]
        wqkv = np.ascontiguousarray(arr.reshape(8, 128, 3, 128).transpose(1, 0, 2, 3))
        in_maps.append(
            {
                "xT": xT,
                "wqkv": wqkv,
                "cosT": cosT,
                "sinT": sinT,
                "rblk": rblk,
                "wout": wout,
                "bias": bias,
                "cmask": cmask,
                "cmask256": cmask256,
                "ident128": ident128,
            }
        )
    return in_maps


def kernel(x, mask, rotary_pos_emb, W_qkv, W_out, b_out):
    if "nc" not in _CACHE:
        _CACHE["nc"] = _build_nc()
    nc = _CACHE["nc"]
    in_maps = _host_prep(x, rotary_pos_emb, W_qkv, W_out, b_out)
    res = run_bass_kernel_spmd(nc, in_maps, core_ids=list(range(NCORES)))
    out = np.empty((B, N, D), dtype=np.float32)
    for c in range(NCORES):
        rows = res.results[c]["out_rows"]  # [B, RPB, D]
        out[0:3, c * RPB : (c + 1) * RPB, :] = rows[0:3]
        # batch 3 used per-q-half exchanges: 128-row chunks per half
        out[3, c * 128 : (c + 1) * 128, :] = rows[3, 0:128]
        out[3, 1024 + c * 128 : 1024 + (c + 1) * 128, :] = rows[3, 128:256]
    return out



# revision 4
# speedup vs baseline: 24.5888x; 24.5888x over previous
"""Trainium2 Bass kernel for nn_Attention_5669356831317.

Dense causal multi-head attention with rotary embeddings on q/k/v:
    qkv = x @ W_qkv ; rotary(q,k,v) ; softmax(causal(q k^T / sqrt(dh))) v ; out @ W_out + b_out

Sharding over 8 NeuronCores:
  - Heads are tensor-parallel: 16 heads / 8 cores = 2 heads per core.
    Each core computes qkv^T for its 2 heads (K=1024 matmul against x^T),
    applies rotary (rotate-half folded into a PE matmul with a signed
    permutation matrix), and runs causal attention for its 8 (batch, head)
    units in a transposed-scores layout: S^T[key, query] so the exp output is
    directly the lhsT-ready P^T, and the softmax denominator comes for free
    from a ones-column appended to V in the P^T @ V matmul.
  - A per-batch AllToAll reshards from head-parallel to row-parallel: each
    core ends with all 1024 inner dims for its 256 rows of each batch, then
    computes its row slice of the output projection (full W_out) + bias.
  - Work is software-pipelined across batches (qkv(b+1) overlaps attention(b)
    overlaps collective(b-1) overlaps projection(b-1)).
  - Host reassembles the row slices.

Host<->device traffic is minimized (the axon tunnel is the wall-clock
bottleneck, ~10-50 MB/s): x / W_qkv / W_out ship as fp16, and the tensors
every core needs (x, W_out, cos/sin tables) are uploaded as 1/8 shards and
reconstructed on-device with AllGathers; outputs come back as fp16. The
on-device pipeline stays float32r except the QKV matmul, which runs
natively in fp16 (its operands are wire-precision anyway).
"""

import numpy as np

import concourse.bass as bass
import concourse.bacc as bacc
import concourse.tile as tile
import concourse.mybir as mybir
from concourse.bass_utils import run_bass_kernel_spmd

B, N, D = 4, 2048, 1024
H, DH = 16, 64
NCORES = 8
ROWS = B * N  # 8192
RPB = N // NCORES  # 256 output rows per (core, batch)
SCALE = DH**-0.5

f32 = mybir.dt.float32
f32r = mybir.dt.float32r
f16 = mybir.dt.float16
AF = mybir.ActivationFunctionType

_CACHE = {}


def _build_nc(single=False):
    nc = bacc.Bacc(
        "TRN2",
        target_bir_lowering=False,
        debug=False,
        num_devices=1 if single else NCORES,
    )

    # per-core 1/8 shard of x^T: chunks [2c, 2c+1] of the [16,128,8,512] layout
    xsh_d = nc.dram_tensor("xsh", [2, 128, 8, 512], f16, kind="ExternalInput")
    wqkv_d = nc.dram_tensor("wqkv", [128, 8, 3, 128], f16, kind="ExternalInput")
    # per-core k-chunk of W_out^T-ish layout: wosh[p, o] = W_out[c*128+p, o]
    wosh_d = nc.dram_tensor("wosh", [128, 1024], f16, kind="ExternalInput")
    # per-core 256-column slice of the [cos; sin] tables ([128, N] each)
    cssh_d = nc.dram_tensor("cssh", [2, 128, 256], f32, kind="ExternalInput")
    rblk_d = nc.dram_tensor("rblk", [128, 128], f32r, kind="ExternalInput")
    bias_d = nc.dram_tensor("bias", [1, D], f32, kind="ExternalInput")
    cmask_d = nc.dram_tensor("cmask", [128, 128], f32, kind="ExternalInput")
    cmask256_d = nc.dram_tensor("cmask256", [128, 256], f32, kind="ExternalInput")
    ident128_d = nc.dram_tensor("ident128", [128, 128], f32, kind="ExternalInput")

    out_d = nc.dram_tensor("out_rows", [B, RPB, D], f16, kind="ExternalOutput")

    shared = "Local" if single else "Shared"
    grp = [list(range(NCORES))]

    with tile.TileContext(nc) as tc:
        with (
            tc.tile_pool(name="const", bufs=1) as const_pool,
            tc.tile_pool(name="big", bufs=1) as big_pool,
            tc.tile_pool(name="xp", bufs=2) as x_pool,
            tc.tile_pool(name="work", bufs=2) as work_pool,
            tc.tile_pool(name="ptp", bufs=3) as pt_pool,
            tc.tile_pool(name="otfp", bufs=1) as otf_pool,
            tc.tile_pool(name="tinyp", bufs=1) as tiny_pool,
            tc.tile_pool(name="ps", bufs=2, space="PSUM") as ps_pool,
            tc.tile_pool(name="psot", bufs=2, space="PSUM") as psot_pool,
            tc.tile_pool(name="dram", bufs=1, space="DRAM") as dram_pool,
        ):
            # ---- ingress: reconstruct shared tensors from 1/8 shards ----
            # stage ExternalInput shards into internal DRAM (collectives
            # cannot read kernel I/O), then AllGather. x first: phase1's
            # first matmuls gate on it.
            xag_in = dram_pool.tile([2, 128, 8, 512], f16, name="xag_in")
            nc.sync.dma_start(xag_in[:], xsh_d[:])
            xag = dram_pool.tile([16, 128, 8, 512], f16, name="xag", addr_space=shared)
            csag_in = dram_pool.tile([2, 128, 256], f32, name="csag_in")
            nc.sync.dma_start(csag_in[:], cssh_d[:])
            csag = dram_pool.tile([8, 2, 128, 256], f32, name="csag", addr_space=shared)
            woag_in = dram_pool.tile([128, 1024], f16, name="woag_in")
            nc.sync.dma_start(woag_in[:], wosh_d[:])
            woag = dram_pool.tile([8, 128, 1024], f16, name="woag", addr_space=shared)
            if single:
                nc.sync.dma_start(xag[0:2], xag_in[:])
                nc.sync.dma_start(csag[0], csag_in[:])
                nc.sync.dma_start(woag[0], woag_in[:])
            else:
                nc.gpsimd.collective_compute(
                    "AllGather", mybir.AluOpType.bypass, replica_groups=grp,
                    ins=[xag_in[:]], outs=[xag[:]],
                )
                nc.gpsimd.collective_compute(
                    "AllGather", mybir.AluOpType.bypass, replica_groups=grp,
                    ins=[csag_in[:]], outs=[csag[:]],
                )
                nc.gpsimd.collective_compute(
                    "AllGather", mybir.AluOpType.bypass, replica_groups=grp,
                    ins=[woag_in[:]], outs=[woag[:]],
                )

            # ---- constants (scalar=ACT HWDGE ring; sync=SP ring) ----
            # wqkv first: phase1's first matmuls gate on it
            wqkv_sb = const_pool.tile([128, 8, 3, 128], f16)
            nc.scalar.dma_start(wqkv_sb[:], wqkv_d[:])
            rblk_sb = const_pool.tile([128, 128], f32r)
            nc.scalar.dma_start(rblk_sb[:], rblk_d[:])
            cosT_sb = const_pool.tile([128, N], f32)
            nc.scalar.dma_start(
                cosT_sb[:].rearrange("p (c n) -> p c n", c=8),
                csag[:, 0].rearrange("c p n -> p c n"),
            )
            sinT_sb = const_pool.tile([128, N], f32)
            nc.scalar.dma_start(
                sinT_sb[:].rearrange("p (c n) -> p c n", c=8),
                csag[:, 1].rearrange("c p n -> p c n"),
            )
            ident128_f = const_pool.tile([128, 128], f32)
            nc.scalar.dma_start(ident128_f[:], ident128_d[:])
            ident128_r = const_pool.tile([128, 128], f32r)
            nc.vector.tensor_copy(ident128_r[:], ident128_f[:])
            cmask_sb = const_pool.tile([128, 128], f32)
            nc.scalar.dma_start(cmask_sb[:], cmask_d[:])
            cmask256_sb = const_pool.tile([128, 256], f32)
            nc.scalar.dma_start(cmask256_sb[:], cmask256_d[:])
            ones_f = const_pool.tile([128, 1], f32)
            nc.vector.memset(ones_f[:], 1.0)
            # deferred: wout gather->convert + bias DMA after phase1(1) (below)
            wout_sb = const_pool.tile([128, 8, D], f32r)
            bias_rep = const_pool.tile([128, D], f32)

            # ---- per-batch activations, rotated through 3 slots each ----
            qT_b, kT_b, vne_b = [], [], []
            for b in range(B):
                qT = big_pool.tile([128, N], f32r, name=f"qT_{b}", tag="qT", bufs=3)
                kT = big_pool.tile([128, N], f32r, name=f"kT_{b}", tag="kT", bufs=3)
                vne = big_pool.tile(
                    [128, 2, 16, 65], f32r, name=f"vne_{b}", tag="vne", bufs=3
                )
                nc.vector.tensor_copy(
                    vne[:, :, :, 64:65], ones_f[:].to_broadcast((128, 2, 16, 1))
                )
                qT_b.append(qT)
                kT_b.append(kT)
                vne_b.append(vne)

            a2a_in_b = [
                dram_pool.tile([8, 128, RPB], f32r, name=f"a2a_in_{b}")
                for b in range(B)
            ]
            a2a_out_b = [
                dram_pool.tile([8, 128, RPB], f32r, name=f"a2a_out_{b}")
                for b in range(B)
            ]
            # last batch exchanges per q-half so the first half's collective
            # fires while the second half's attention still runs
            a2a_in3 = [
                dram_pool.tile([8, 128, 128], f32r, name=f"a2a_in3_{qh}")
                for qh in range(2)
            ]
            a2a_out3 = [
                dram_pool.tile([8, 128, 128], f32r, name=f"a2a_out3_{qh}")
                for qh in range(2)
            ]

            def phase1_gen(b):
                """qkv^T + rotary for batch b; yields after each 512-chunk."""
                for jj in range(4):  # 512-wide chunks within the batch
                    j = b * 4 + jj
                    cosc = cosT_sb[:, jj * 512 : (jj + 1) * 512]
                    sinc = sinT_sb[:, jj * 512 : (jj + 1) * 512]
                    acA = ps_pool.tile([128, 1024], f32, tag="ps", name="acA")
                    acB = ps_pool.tile([128, 1024], f32, tag="ps", name="acB")
                    # accumulation regions: q=acA[0:512], k=acA[512:1024], v=acB[0:512]
                    regions = [acA[:, 0:512], acA[:, 512:1024], acB[:, 0:512]]
                    x8 = x_pool.tile([128, 8, 512], f16, tag="x8")
                    if j == 0:
                        # split the very first chunk across both rings so the
                        # first matmuls start as early as possible
                        nc.sync.dma_start(x8[:, 0:4, :], xag[0, :, 0:4, :])
                        nc.scalar.dma_start(x8[:, 4:8, :], xag[0, :, 4:8, :])
                    else:
                        eng = nc.sync if j % 2 == 0 else nc.scalar
                        eng.dma_start(x8[:], xag[j])
                    for k in range(8):
                        for m in range(3):
                            nc.tensor.matmul(
                                regions[m],
                                wqkv_sb[:, k, m, :],
                                x8[:, k, :],
                                start=(k == 0),
                                stop=(k == 7),
                            )
                    vrot = None
                    for m in range(3):  # q, k, v
                        raw = work_pool.tile([128, 512], f32r, tag="raw")
                        nc.scalar.copy(raw[:], regions[m])  # evacuate+round (ACT)
                        rot = acB[:, 512:1024]  # rotate-half scratch bank
                        nc.tensor.matmul(rot, rblk_sb[:], raw[:], start=True, stop=True)
                        tmp = work_pool.tile([128, 512], f32, tag="tmp")
                        nc.vector.tensor_mul(tmp[:], rot, sinc)
                        if m < 2:
                            dest = (qT_b[b] if m == 0 else kT_b[b])[
                                :, jj * 512 : (jj + 1) * 512
                            ]
                            nc.gpsimd.tensor_mul(dest, raw[:], cosc)
                            nc.vector.tensor_add(dest, dest, tmp[:])
                        else:
                            vrot = work_pool.tile([128, 512], f32r, tag="vrot")
                            nc.gpsimd.tensor_mul(vrot[:], raw[:], cosc)
                            nc.vector.tensor_add(vrot[:], vrot[:], tmp[:])
                    # transpose v' into normal layout; each [128,128] transpose
                    # yields both heads' [n, dh] blocks side by side
                    vt_ps = ps_pool.tile([128, 1024], f32r, tag="ps", name="vt_ps")
                    for t in range(4):
                        nc.tensor.transpose(
                            vt_ps[:, t * 256 : t * 256 + 128],
                            vrot[:, t * 128 : (t + 1) * 128],
                            ident128_r[:],
                        )
                    for t in range(4):
                        jb = jj * 4 + t
                        nc.vector.tensor_copy(
                            vne_b[b][:, :, jb, 0:64],
                            vt_ps[:, t * 256 : t * 256 + 128].rearrange(
                                "p (h d) -> p h d", h=2
                            ),
                        )
                    yield

            def attn_gen(b, qh_hook=None):
                """Causal attention for batch b; both head-halves advance
                together so their K=64 scores matmuls occupy disjoint PE
                row-groups concurrently. Yields after each jb step."""
                for qh in range(2):
                    qbase = qh * 1024
                    OTs = [
                        psot_pool.tile([65, 1024], f32, tag="ot", name=f"OT_{hh}")
                        for hh in range(2)
                    ]
                    jb_max = 8 * qh + 7
                    for jb in range(jb_max + 1):
                        w0 = max(0, jb * 128 - qbase)
                        # fp32r matmuls run 4x slower below 256 columns: widen
                        # a 128-wide diagonal partial to 256 and zero the extra
                        # 128 invalid columns with the extended causal mask
                        widen = jb * 128 > qbase and (jb * 128 - qbase) % 512 == 384
                        w0e = w0 - 128 if widen else w0

                        def _ranges():
                            for sc in range(2):
                                clo = qbase + sc * 512
                                chi = clo + 512
                                lo = max(clo, jb * 128)
                                if lo >= chi:
                                    continue
                                if chi - lo == 128:
                                    lo -= 128
                                yield sc, lo, chi

                        sts = [
                            ps_pool.tile([128, 1024], f32, tag="ps", name=f"st_{hh}")
                            for hh in range(2)
                        ]
                        # alternate head-halves so consecutive matmuls land on
                        # different PE row-groups (base partitions 0 / 64)
                        for sc, lo, chi in _ranges():
                            for hh in range(2):
                                hsl = slice(hh * 64, (hh + 1) * 64)
                                nc.tensor.matmul(
                                    sts[hh][:, lo - qbase : chi - qbase],
                                    kT_b[b][hsl, jb * 128 : (jb + 1) * 128],
                                    qT_b[b][hsl, lo:chi],
                                    start=True,
                                    stop=True,
                                )
                        for hh in range(2):
                            pt = pt_pool.tile([128, 1024], f32r, tag="pt")
                            nc.scalar.activation(
                                pt[:, w0e:1024], sts[hh][:, w0e:1024], AF.Exp, scale=SCALE
                            )
                            if jb * 128 >= qbase:
                                # zero below-diagonal keys (and the widened
                                # invalid columns, if any)
                                if widen:
                                    nc.vector.tensor_mul(
                                        pt[:, w0e : w0e + 256],
                                        pt[:, w0e : w0e + 256],
                                        cmask256_sb[:],
                                    )
                                else:
                                    nc.vector.tensor_mul(
                                        pt[:, w0 : w0 + 128],
                                        pt[:, w0 : w0 + 128],
                                        cmask_sb[:],
                                    )
                            vw = vne_b[b][:, hh, jb, :]
                            for sc, lo, chi in _ranges():
                                nc.tensor.matmul(
                                    OTs[hh][:, lo - qbase : chi - qbase],
                                    vw,
                                    pt[:, lo - qbase : chi - qbase],
                                    start=(jb == 0),
                                    stop=(jb == 8 * qh + 4 * sc + 3),
                                )
                        yield
                    # normalize by the ones-column sums, stage into qT_b[b]
                    for hh in range(2):
                        hsl = slice(hh * 64, (hh + 1) * 64)
                        gsl = slice(qbase, qbase + 1024)
                        rep = tiny_pool.tile([64, 1024], f32, tag="rep")
                        nc.vector.reciprocal(rep[0:1, :], OTs[hh][64:65, :])
                        nc.gpsimd.partition_broadcast(rep[:], rep[0:1, :], channels=64)
                        nc.vector.tensor_mul(
                            qT_b[b][hsl, gsl], OTs[hh][0:64, :], rep[:]
                        )
                    if qh_hook is not None:
                        qh_hook(qh)

            def stage(b):
                """Ship batch b's attention output through the AllToAll."""
                nc.sync.dma_start(
                    a2a_in_b[b][:].rearrange("t p r -> p t r"),
                    qT_b[b][:].rearrange("p (t r) -> p t r", t=8),
                )
                if single:
                    nc.sync.dma_start(a2a_out_b[b][:], a2a_in_b[b][:])
                else:
                    nc.gpsimd.collective_compute(
                        "AllToAll",
                        mybir.AluOpType.bypass,
                        replica_groups=grp,
                        ins=[a2a_in_b[b][:]],
                        outs=[a2a_out_b[b][:]],
                    )

            def stage3_half(qh):
                nc.sync.dma_start(
                    a2a_in3[qh][:].rearrange("t p r -> p t r"),
                    qT_b[3][:, qh * 1024 : (qh + 1) * 1024].rearrange(
                        "p (t r) -> p t r", t=8
                    ),
                )
                if single:
                    nc.sync.dma_start(a2a_out3[qh][:], a2a_in3[qh][:])
                else:
                    nc.gpsimd.collective_compute(
                        "AllToAll",
                        mybir.AluOpType.bypass,
                        replica_groups=grp,
                        ins=[a2a_in3[qh][:]],
                        outs=[a2a_out3[qh][:]],
                    )

            def proj_gen(b):
                """Output projection for this core's 256 rows of batch b, in
                self-contained per-row-chunk pieces so it can interleave into
                attention."""
                otf2 = otf_pool.tile([128, 8, RPB], f32r, tag="otf")
                if b == 3:
                    for qh in range(2):
                        nc.sync.dma_start(
                            otf2[:, :, qh * 128 : (qh + 1) * 128],
                            a2a_out3[qh][:].rearrange("i p r -> p i r"),
                        )
                else:
                    nc.sync.dma_start(
                        otf2[:], a2a_out_b[b][:].rearrange("i p r -> p i r")
                    )
                yield
                for rr in range(2):
                    ps = ps_pool.tile([128, 1024], f32, tag="ps", name=f"pp_{rr}")
                    for k in range(8):
                        for n_ in range(2):
                            nc.tensor.matmul(
                                ps[:, n_ * 512 : (n_ + 1) * 512],
                                otf2[:, k, rr * 128 : (rr + 1) * 128],
                                wout_sb[:, k, n_ * 512 : (n_ + 1) * 512],
                                start=(k == 0),
                                stop=(k == 7),
                            )
                    for n_ in range(2):
                        res = work_pool.tile([128, 512], f32, tag="tmp")
                        nc.vector.tensor_add(
                            res[:],
                            ps[:, n_ * 512 : (n_ + 1) * 512],
                            bias_rep[:, n_ * 512 : (n_ + 1) * 512],
                        )
                        res16 = work_pool.tile([128, 512], f16, tag="res16")
                        nc.gpsimd.tensor_copy(res16[:], res[:])
                        nc.scalar.dma_start(
                            out_d[
                                b,
                                rr * 128 : (rr + 1) * 128,
                                n_ * 512 : (n_ + 1) * 512,
                            ],
                            res16[:],
                        )
                    yield

            # software pipeline across batches: attention(b) is interleaved
            # with phase1(b+1) at (jb-step, chunk) granularity so the PE
            # absorbs the ACT exp-throughput deficit.
            def run_all(gen):
                for _ in gen:
                    pass

            def interleave(attn_g, p1_g, every=10):
                i = 0
                for _ in attn_g:
                    i += 1
                    if p1_g is not None and i % every == 0:
                        next(p1_g, None)
                if p1_g is not None:
                    run_all(p1_g)

            run_all(phase1_gen(0))
            run_all(phase1_gen(1))
            # projection weights arrive while attention runs: gather shards
            # already live in woag; convert fp16 -> f32r per k-chunk
            for k in range(8):
                wo16 = work_pool.tile([128, 1024], f16, tag="wo16")
                nc.scalar.dma_start(wo16[:], woag[k])
                nc.vector.tensor_copy(wout_sb[:, k, :], wo16[:])
            nc.scalar.dma_start(bias_rep[:], bias_d[:].to_broadcast((128, D)))
            interleave(attn_gen(0), phase1_gen(2))
            stage(0)
            interleave(attn_gen(1), phase1_gen(3))
            stage(1)
            run_all(proj_gen(0))
            interleave(attn_gen(2), proj_gen(1), every=8)
            stage(2)
            interleave(attn_gen(3, qh_hook=stage3_half), proj_gen(2), every=8)
            run_all(proj_gen(3))

    nc.compile()
    return nc


def _host_prep(x, rotary_pos_emb, W_qkv, W_out, b_out):
    x = np.asarray(x, dtype=np.float32)
    W_qkv = np.asarray(W_qkv, dtype=np.float32)
    W_out = np.asarray(W_out, dtype=np.float32)
    b_out = np.asarray(b_out, dtype=np.float32)
    rot = np.asarray(rotary_pos_emb, dtype=np.float32)

    xf = x.reshape(ROWS, D)
    # [16, 128, 8, 512]: chunk j, partition p, k-chunk, col n -> xf[j*512+n, k*128+p]
    xT16 = np.ascontiguousarray(
        xf.reshape(16, 512, 8, 128).transpose(0, 3, 2, 1).astype(np.float16)
    )

    cos = np.cos(rot).T  # [64, 2048]
    sin = np.sin(rot).T
    cosT = np.tile(cos, (2, 1))  # [128, 2048]
    sinT = np.tile(sin, (2, 1))
    # [8, 2, 128, 256]: core c's 256-column slice of [cos; sin]
    cs = np.ascontiguousarray(
        np.stack([cosT, sinT], axis=0)  # [2, 128, 2048]
        .reshape(2, 128, 8, 256)
        .transpose(2, 0, 1, 3)
    )

    # rotate_half as a matrix: (R t)[2i] = -t[2i+1], (R t)[2i+1] = t[2i]
    R64 = np.zeros((64, 64), np.float32)
    idx = np.arange(0, 64, 2)
    R64[idx, idx + 1] = -1.0
    R64[idx + 1, idx] = 1.0
    rblk = np.zeros((128, 128), np.float32)
    rblk[0:64, 0:64] = R64.T
    rblk[64:128, 64:128] = R64.T

    cmask = (np.arange(128)[:, None] <= np.arange(128)[None, :]).astype(np.float32)
    cmask256 = np.concatenate([np.zeros((128, 128), np.float32), cmask], axis=1)
    ident128 = np.eye(128, dtype=np.float32)

    W_out16 = W_out.astype(np.float16)  # [1024, 1024]; core c ships rows c*128..+127
    bias = np.ascontiguousarray(b_out.reshape(1, D))

    in_maps = []
    for c in range(NCORES):
        hsl = slice(2 * c * 64, (2 * c + 2) * 64)
        Wq = W_qkv[:, 0 * D : 1 * D][:, hsl]
        Wk = W_qkv[:, 1 * D : 2 * D][:, hsl]
        Wv = W_qkv[:, 2 * D : 3 * D][:, hsl]
        arr = np.stack([Wq, Wk, Wv], axis=1)  # [1024, 3, 128]
        wqkv = np.ascontiguousarray(
            arr.reshape(8, 128, 3, 128).transpose(1, 0, 2, 3).astype(np.float16)
        )
        in_maps.append(
            {
                "xsh": xT16[2 * c : 2 * c + 2],
                "wqkv": wqkv,
                "wosh": W_out16[c * 128 : (c + 1) * 128],
                "cssh": cs[c],
                "rblk": rblk,
                "bias": bias,
                "cmask": cmask,
                "cmask256": cmask256,
                "ident128": ident128,
            }
        )
    return in_maps


def kernel(x, mask, rotary_pos_emb, W_qkv, W_out, b_out):
    if "nc" not in _CACHE:
        _CACHE["nc"] = _build_nc()
    nc = _CACHE["nc"]
    in_maps = _host_prep(x, rotary_pos_emb, W_qkv, W_out, b_out)
    res = run_bass_kernel_spmd(nc, in_maps, core_ids=list(range(NCORES)))
    out = np.empty((B, N, D), dtype=np.float32)
    for c in range(NCORES):
        rows = res.results[c]["out_rows"].astype(np.float32)  # [B, RPB, D]
        out[0:3, c * RPB : (c + 1) * RPB, :] = rows[0:3]
        # batch 3 used per-q-half exchanges: 128-row chunks per half
        out[3, c * 128 : (c + 1) * 128, :] = rows[3, 0:128]
        out[3, 1024 + c * 128 : 1024 + (c + 1) * 128, :] = rows[3, 128:256]
    return out
